# revision 18
# baseline (speedup 1.0000x reference)
"""CrossScaleConvolutionalAttentionDenoising Trainium2 Bass kernel.

Data-parallel over batch: B=16 images, 8 NeuronCores, 2 images per core.
All params replicated; BN folded into conv weights/biases on the host.
Layout: channels on partitions, pixels (h*64+w) on the free dim.
  - 3 depthwise convs (k=3,5,7) + bias -> SiLU          (branch scales)
  - 1x1 conv 768->256 (PE matmul) + bias -> SiLU -> "sigmoid" (channel attn)
  - channel mean/max of fused -> 3x3 conv (2->1) -> "sigmoid"  (spatial attn)
  - out = attn_w * (s0+s1+s2) + attn2 * x
sigmoid(z) is computed as 0.5*(tanh(z/2)+1) so the whole kernel uses only the
silu_and_others ACT table set (silu + tanh) -- no table reloads.
"""

from contextlib import ExitStack

import numpy as np

import concourse.bacc as bacc
import concourse.bass_isa as bass_isa
import concourse.tile as tile
from concourse import mybir
from concourse.bass_utils import run_bass_kernel_spmd

N_CORES = 8
B, C, H, W = 16, 256, 64, 64
BPC = B // N_CORES  # images per core
KS = (3, 5, 7)
EPS = 1e-5
PX = H * W  # 4096
PXC = 512   # pixel chunk
F32 = mybir.dt.float32
BF16 = mybir.dt.bfloat16
AF = mybir.ActivationFunctionType
OP = mybir.AluOpType
RED = bass_isa.ReduceOp

_CACHE = {}


def _taps(k):
    p = k // 2
    for dh in range(k):
        for dw in range(k):
            yield 3 - p + dh, 3 - p + dw  # slice offsets into the 70x70 pad


def build(tap_avg, tap_max, bias_u):
    nc = bacc.Bacc("TRN2", target_bir_lowering=False, debug=False)
    x_d = nc.dram_tensor("x", [BPC, C, H, W], F32, kind="ExternalInput")
    dww_d = nc.dram_tensor("dww", [128, 2, 83], F32, kind="ExternalInput")
    dwb_d = nc.dram_tensor("dwb", [128, 2, 3], F32, kind="ExternalInput")
    aw_d = nc.dram_tensor("aw", [128, 6, 2, 128], BF16, kind="ExternalInput")
    ba_d = nc.dram_tensor("ba", [128, 2], F32, kind="ExternalInput")
    out_d = nc.dram_tensor("out", [BPC, C, H, W], F32, kind="ExternalOutput")
    st_d = nc.dram_tensor("stage", [BPC, PX], BF16)  # spatial-attn tanh map

    with tile.TileContext(nc) as tc, ExitStack() as ctx:
        const = ctx.enter_context(tc.tile_pool(name="const", bufs=1))
        dww = const.tile([128, 2, 83], F32)
        nc.sync.dma_start(out=dww, in_=dww_d[:, :, :])
        dwb = const.tile([128, 2, 3], F32)
        nc.sync.dma_start(out=dwb, in_=dwb_d[:, :, :])
        aw = const.tile([128, 6, 2, 128], BF16)
        nc.sync.dma_start(out=aw, in_=aw_d[:, :, :, :])
        ba = const.tile([128, 2], F32)
        nc.sync.dma_start(out=ba, in_=ba_d[:, :])

        scl_p = ctx.enter_context(tc.tile_pool(name="scl", bufs=12))
        scales = {}

        # ---- phase 1+2: depthwise convs, streamed per (img, channel-group) ----
        with tc.tile_pool(name="xpad", bufs=2) as xpool, \
             tc.tile_pool(name="dwacc", bufs=2) as apool:
            for b in range(BPC):
                for g in range(2):
                    xp = xpool.tile([128, 70, 70], F32, tag="xpad")
                    nc.vector.memset(xp, 0.0)
                    nc.sync.dma_start(
                        out=xp[:, 3:67, 3:67],
                        in_=x_d[b, g * 128:(g + 1) * 128, :, :])
                    tbase = 0
                    for br, k in enumerate(KS):
                        acc = apool.tile([128, 64, 64], F32, tag="acc")
                        for i, (oh, ow) in enumerate(_taps(k)):
                            w_ap = dww[:, g, tbase + i:tbase + i + 1]
                            src = xp[:, oh:oh + 64, ow:ow + 64]
                            if i == 0:
                                nc.vector.tensor_scalar_mul(acc, src, w_ap)
                            else:
                                nc.vector.scalar_tensor_tensor(
                                    acc, src, w_ap, acc, OP.mult, OP.add)
                        tbase += k * k
                        # silu(z) = z * sigmoid(z), z = acc + bias
                        sg = apool.tile([128, PX], BF16, tag="sg")
                        nc.scalar.activation(
                            sg, acc.rearrange("p a b -> p (a b)"), AF.Sigmoid,
                            bias=dwb[:, g, br:br + 1], scale=1.0)
                        s = scl_p.tile([128, PX], BF16, tag="scl")
                        nc.vector.scalar_tensor_tensor(
                            s, acc.rearrange("p a b -> p (a b)"),
                            dwb[:, g, br:br + 1], sg, OP.add, OP.mult)
                        scales[(b, g, br)] = s

        # persistent mid-pipeline tensors (opened after dw pools closed)
        sum3_p = ctx.enter_context(tc.tile_pool(name="sum3", bufs=4))
        sum3 = {}

        # ---- phase 3: sum/max over branches+channels, spatial attention ----
        with tc.tile_pool(name="p3", bufs=2) as p3, \
             tc.tile_pool(name="sp", bufs=6) as sp:
            for b in range(BPC):
                pads = {}
                for g in range(2):
                    s0, s1, s2 = (scales[(b, g, i)] for i in range(3))
                    t01 = p3.tile([128, PX], BF16, tag="tmp")
                    nc.vector.tensor_tensor(t01, s0, s1, OP.add)
                    s3 = sum3_p.tile([128, PX], BF16, tag="sum3")
                    nc.vector.tensor_tensor(s3, t01, s2, OP.add)
                    sum3[(b, g)] = s3
                    m01 = p3.tile([128, PX], BF16, tag="tmp")
                    nc.vector.tensor_tensor(m01, s0, s1, OP.max)
                    mx = p3.tile([128, PX], BF16, tag="mx")
                    nc.vector.tensor_tensor(mx, m01, s2, OP.max)
                    # reduce across the 128 partitions (channels)
                    ar = p3.tile([128, PX], BF16, tag="ar")
                    nc.gpsimd.partition_all_reduce(ar, s3, 128, RED.add)
                    mr = p3.tile([128, PX], BF16, tag="ar")
                    nc.gpsimd.partition_all_reduce(mr, mx, 128, RED.max)
                    pa = sp.tile([64, 64], BF16, tag="pads")
                    nc.sync.dma_start(out=pa, in_=ar[0:1, :])
                    pm = sp.tile([64, 64], BF16, tag="pads")
                    nc.sync.dma_start(out=pm, in_=mr[0:1, :])
                    pads[(g, "a")], pads[(g, "m")] = pa, pm
                # combine groups into [64, 64] maps (base partition 0)
                cmba = sp.tile([64, 64], F32, tag="cmba")
                cmbm = sp.tile([64, 64], F32, tag="cmbm")
                nc.vector.tensor_tensor(cmba, pads[(0, "a")], pads[(1, "a")], OP.add)
                nc.vector.tensor_tensor(cmbm, pads[(0, "m")], pads[(1, "m")], OP.max)
                # engines cannot address partition offsets != 0 mod 32, so
                # build h-shifted copies via DMA: sh[d][h, 1+w] = map[h+d, w]
                sh = {}
                for ci, cmb in ((0, cmba), (1, cmbm)):
                    for d in (-1, 0, 1):
                        t = sp.tile([64, 66], F32, tag="shift")
                        nc.vector.memset(t, 0.0)
                        n = 64 - abs(d)
                        nc.sync.dma_start(
                            out=t[max(0, -d):max(0, -d) + n, 1:65],
                            in_=cmb[max(0, d):max(0, d) + n, :])
                        sh[(ci, d)] = t
                # 3x3 conv over (avg, max) with folded taps
                acc2 = sp.tile([64, 64], F32, tag="acc2")
                first = True
                for ci, taps in ((0, tap_avg), (1, tap_max)):
                    for dh in range(3):
                        for dw in range(3):
                            src = sh[(ci, dh - 1)][:, dw:dw + 64]
                            wv = float(taps[dh, dw])
                            if first:
                                nc.vector.tensor_scalar_mul(acc2, src, wv)
                                first = False
                            else:
                                nc.vector.scalar_tensor_tensor(
                                    acc2, src, wv, acc2, OP.mult, OP.add)
                tu = sp.tile([64, 64], BF16, tag="tu")
                bu = sp.tile([64, 1], F32, tag="bu")
                nc.vector.memset(bu, bias_u)
                nc.scalar.activation(tu, acc2, AF.Sigmoid, bias=bu, scale=1.0)
                nc.sync.dma_start(out=st_d[b, :], in_=tu)

        # ---- phase 4: 1x1 conv (768->256) -> SiLU -> tanh-half ----
        th_p = ctx.enter_context(tc.tile_pool(name="th", bufs=4))
        th = {}
        with tc.tile_pool(name="ps", bufs=4, space="PSUM") as psp, \
             tc.tile_pool(name="p4", bufs=4) as p4:
            for b in range(BPC):
                for m in range(2):
                    tht = th_p.tile([128, PX], BF16, tag="th")
                    th[(b, m)] = tht
                for pxc in range(PX // PXC):
                    c0 = pxc * PXC
                    for m in range(2):
                        ps = psp.tile([128, PXC], F32, tag="ps")
                        for kc in range(6):
                            br, g = kc // 2, kc % 2
                            nc.tensor.matmul(
                                ps, aw[:, kc, m, :],
                                scales[(b, g, br)][:, c0:c0 + PXC],
                                start=(kc == 0), stop=(kc == 5))
                        # t = silu(ps + ba); attn = sigmoid(t)
                        sg1 = p4.tile([128, PXC], BF16, tag="sg1")
                        nc.scalar.activation(
                            sg1, ps, AF.Sigmoid, bias=ba[:, m:m + 1], scale=1.0)
                        tt = p4.tile([128, PXC], BF16, tag="tt")
                        nc.vector.scalar_tensor_tensor(
                            tt, ps, ba[:, m:m + 1], sg1, OP.add, OP.mult)
                        nc.scalar.activation(
                            th[(b, m)][:, c0:c0 + PXC], tt, AF.Sigmoid, scale=1.0)

        # ---- phase 6: out = 0.5*((th+1)*sum3 + (tu+1)*x) ----
        with tc.tile_pool(name="p6", bufs=4) as p6:
            for b in range(BPC):
                for g in range(2):
                    for pxc in range(PX // PXC):
                        c0 = pxc * PXC
                        r0 = c0 // 64
                        xa = p6.tile([128, 8, 64], F32, tag="xa")
                        nc.sync.dma_start(
                            out=xa,
                            in_=x_d[b, g * 128:(g + 1) * 128, r0:r0 + 8, :])
                        u1c = p6.tile([128, PXC], BF16, tag="u1c")
                        nc.sync.dma_start(
                            out=u1c,
                            in_=st_d[b:b + 1, c0:c0 + PXC].to_broadcast((128, PXC)))
                        q = p6.tile([128, PXC], F32, tag="q")
                        nc.vector.tensor_tensor(
                            q, u1c, xa.rearrange("p a b -> p (a b)"), OP.mult)
                        pp = p6.tile([128, PXC], F32, tag="pp")
                        nc.vector.tensor_tensor(
                            pp, th[(b, g)][:, c0:c0 + PXC],
                            sum3[(b, g)][:, c0:c0 + PXC], OP.mult)
                        ot = p6.tile([128, 8, 64], F32, tag="ot")
                        nc.vector.tensor_tensor(
                            ot.rearrange("p a b -> p (a b)"), pp, q, OP.add)
                        nc.sync.dma_start(
                            out=out_d[b, g * 128:(g + 1) * 128, r0:r0 + 8, :],
                            in_=ot)
    nc.compile()
    return nc


def _prep(inputs):
    """Host-side folding of BN/bias into conv weights; builds per-core input maps."""
    f = lambda a: np.asarray(a, dtype=np.float32)
    x = f(inputs["x"])
    dw_w = [f(w) for w in inputs["dw_w"]]
    dw_b, bn_g, bn_b = f(inputs["dw_b"]), f(inputs["bn_g"]), f(inputs["bn_b"])
    bn_m, bn_v = f(inputs["bn_m"]), f(inputs["bn_v"])
    aw, ab = f(inputs["aw"]), f(inputs["ab"])
    a_g, a_b = f(inputs["a_g"]), f(inputs["a_b"])
    a_m, a_v = f(inputs["a_m"]), f(inputs["a_v"])
    sq_w, sq_b = f(inputs["sq_w"]), f(inputs["sq_b"])

    s = bn_g / np.sqrt(bn_v + EPS)                  # (3, C)
    biasdw = (dw_b - bn_m) * s + bn_b               # (3, C)
    dww = np.zeros((128, 2, 83), np.float32)
    dwb = np.zeros((128, 2, 3), np.float32)
    tb = 0
    for br, k in enumerate(KS):
        wk = (dw_w[br][:, :, 0, :] * s[br][None, None, :]).reshape(k * k, C)
        for g in range(2):
            dww[:, g, tb:tb + k * k] = wk[:, g * 128:(g + 1) * 128].T
            dwb[:, g, br] = biasdw[br, g * 128:(g + 1) * 128]
        tb += k * k

    sa = a_g / np.sqrt(a_v + EPS)                   # (256,)
    awf = aw[0, 0] * sa[None, :]                    # (768, 256)
    awt = np.zeros((128, 6, 2, 128), np.float32)
    for kc in range(6):
        for m in range(2):
            awt[:, kc, m, :] = awf[kc * 128:(kc + 1) * 128, m * 128:(m + 1) * 128]
    ba2 = ((ab - a_m) * sa + a_b).reshape(2, 128).T.copy()   # (128, 2)

    tap_avg = sq_w[:, :, 0, 0] / 768.0
    tap_max = sq_w[:, :, 1, 0]
    bias_u = float(sq_b[0])

    import ml_dtypes
    awt_bf = awt.astype(ml_dtypes.bfloat16)
    in_maps = []
    for core in range(N_CORES):
        in_maps.append({
            "x": np.ascontiguousarray(x[core * BPC:(core + 1) * BPC]),
            "dww": dww, "dwb": dwb, "aw": awt_bf, "ba": ba2,
        })
    return in_maps, (tap_avg, tap_max, bias_u)


def kernel(**inputs):
    in_maps, spatial = _prep(inputs)
    key = (spatial[0].tobytes(), spatial[1].tobytes(), spatial[2])
    if _CACHE.get("key") != key:
        _CACHE["nc"] = build(*spatial)
        _CACHE["key"] = key
    res = run_bass_kernel_spmd(_CACHE["nc"], in_maps, core_ids=list(range(N_CORES)))
    out = np.concatenate([r["out"] for r in res.results], axis=0)
    return np.ascontiguousarray(out, dtype=np.float32)


# revision 29
# speedup vs baseline: 2.1610x; 2.1610x over previous
"""CrossScaleConvolutionalAttentionDenoising Trainium2 Bass kernel.

Data-parallel over batch: B=16 images, 8 NeuronCores, 2 images per core.
All params replicated; BN folded into conv weights/biases on the host.
Layout: channels on partitions, pixels (h*64+w) on the free dim.
  - 3 depthwise convs (k=3,5,7) + bias -> SiLU          (branch scales)
  - 1x1 conv 768->256 (PE matmul) + bias -> SiLU -> "sigmoid" (channel attn)
  - channel mean/max of fused -> 3x3 conv (2->1) -> "sigmoid"  (spatial attn)
  - out = attn_w * (s0+s1+s2) + attn2 * x
sigmoid(z) is computed as 0.5*(tanh(z/2)+1) so the whole kernel uses only the
silu_and_others ACT table set (silu + tanh) -- no table reloads.
"""

from contextlib import ExitStack

import numpy as np

import concourse.bacc as bacc
import concourse.bass_isa as bass_isa
import concourse.tile as tile
from concourse import mybir
from concourse.bass_utils import run_bass_kernel_spmd

N_CORES = 8
B, C, H, W = 16, 256, 64, 64
BPC = B // N_CORES  # images per core
KS = (3, 5, 7)
EPS = 1e-5
PX = H * W  # 4096
PXC = 512   # pixel chunk
F32 = mybir.dt.float32
BF16 = mybir.dt.bfloat16
AF = mybir.ActivationFunctionType
OP = mybir.AluOpType
RED = bass_isa.ReduceOp

_CACHE = {}


def _taps(k):
    p = k // 2
    for dh in range(k):
        for dw in range(k):
            yield 3 - p + dh, 3 - p + dw  # slice offsets into the 70x70 pad


def build(tap_avg, tap_max, bias_u):
    nc = bacc.Bacc("TRN2", target_bir_lowering=False, debug=False)
    x_d = nc.dram_tensor("x", [BPC, C, H, W], F32, kind="ExternalInput")
    # per-channel banded-Toeplitz stationaries: [pair, tap(br,delta), c2, h_in, h_out]
    sta_d = nc.dram_tensor("sta", [128, 15, 2, 64, 64], BF16, kind="ExternalInput")
    dwb_d = nc.dram_tensor("dwb", [128, 2, 3], F32, kind="ExternalInput")
    aw_d = nc.dram_tensor("aw", [128, 6, 2, 128], BF16, kind="ExternalInput")
    ba_d = nc.dram_tensor("ba", [128, 2], F32, kind="ExternalInput")
    out_d = nc.dram_tensor("out", [BPC, C, H, W], F32, kind="ExternalOutput")
    st_d = nc.dram_tensor("stage", [BPC, PX], BF16)  # spatial-attn sigmoid map
    zd_d = nc.dram_tensor("zstage", [BPC, 2, 3, 128, H, W], BF16)  # dw reassembly

    with tile.TileContext(nc) as tc, ExitStack() as ctx:
        const = ctx.enter_context(tc.tile_pool(name="const", bufs=1))
        dwb = const.tile([128, 2, 3], F32)
        nc.sync.dma_start(out=dwb, in_=dwb_d[:, :, :])
        aw = const.tile([128, 6, 2, 128], BF16)
        nc.sync.dma_start(out=aw, in_=aw_d[:, :, :, :])
        ba = const.tile([128, 2], F32)
        nc.sync.dma_start(out=ba, in_=ba_d[:, :])

        scl_p = ctx.enter_context(tc.tile_pool(name="scl", bufs=12))
        scales = {}
        # z-accumulators in channel-major layout, reused in place for silu
        for b in range(BPC):
            for g in range(2):
                for br in range(3):
                    zt = scl_p.tile([128, PX], BF16, tag="scl")
                    scales[(b, g, br)] = zt

        # ---- phase 1+2: depthwise convs on the PE (banded Toeplitz over h) ----
        # xb: [(c2,h)=128, (b, pair, w_pad)] bf16, w zero-padded by 3 each side
        with tc.tile_pool(name="xb", bufs=1) as xbp, \
             tc.tile_pool(name="stg", bufs=2) as stgp, \
             tc.tile_pool(name="zst", bufs=6) as zstp, \
             tc.tile_pool(name="psd", bufs=2, space="PSUM") as psd:
            xb = xbp.tile([128, BPC, 128, 70], BF16)
            nc.vector.memset(xb, 0.0)
            for b in range(BPC):
                for ph in range(2):  # split to stay under DMA descriptor limit
                    nc.gpsimd.dma_start(  # gpsimd DMA casts f32 -> bf16
                        out=xb[:, b, ph * 64:(ph + 1) * 64, 3:67],
                        in_=x_d[b, ph * 128:(ph + 1) * 128].rearrange(
                            "(pr c2) h w -> (c2 h) pr w", c2=2))
            tslot = {}  # (br, dlt) -> (t index, w offset)
            t0 = 0
            for br, k in enumerate(KS):
                p = k // 2
                for dlt in range(k):
                    tslot[(br, dlt)] = (t0 + dlt, 3 - p + dlt)
                t0 += k
            for q in range(32):  # 4 pairs per chunk
                stg = stgp.tile([128, 4, 15, 64], BF16, tag="stg")
                for pl in range(4):
                    nc.sync.dma_start(
                        out=stg[:, pl, :, :],
                        in_=sta_d[4 * q + pl].rearrange(
                            "t c2 hi ho -> (c2 hi) t ho"))
                pss = []
                for br in range(3):
                    pst = psd.tile([128, 512], F32, tag=f"ps{br}")
                    pss.append(pst)
                for pl in range(4):
                    pair = 4 * q + pl
                    for br, k in enumerate(KS):
                        for c2 in range(2):
                            lo = c2 * 64
                            for dlt in range(k):
                                t, off = tslot[(br, dlt)]
                                nc.tensor.matmul(
                                    pss[br][lo:lo + 64, pl * 128:(pl + 1) * 128],
                                    stg[lo:lo + 64, pl, t, :],
                                    xb[lo:lo + 64, :, pair, off:off + 64],
                                    start=(dlt == 0), stop=(dlt == k - 1),
                                    tile_position=(lo, lo))
                # copy PSUM z-values to bf16 staging, then scatter to DRAM in
                # channel-major order (SBUF APs must stay partition-leading,
                # so the layout change happens on the DRAM side)
                g, pb = q // 16, (q % 16) * 4
                for br in range(3):
                    zs = zstp.tile([128, 4, BPC, 64], BF16, tag="zst")
                    if br % 2 == 0:
                        nc.vector.tensor_copy(
                            zs.rearrange("p a b c -> p (a b c)"), pss[br])
                    else:
                        nc.scalar.copy(
                            zs.rearrange("p a b c -> p (a b c)"), pss[br])
                    for b in range(BPC):
                        # dst iter (c2, h, pl, w) matches src partition-major
                        dst = zd_d[b, g, br, 2 * pb:2 * pb + 8].rearrange(
                            "(pl c2) h w -> (c2 h) pl w", c2=2)
                        nc.sync.dma_start(out=dst, in_=zs[:, :, b, :])
        # gather: one big linear DMA per channel-major z tile
        for b in range(BPC):
            for g in range(2):
                for br in range(3):
                    nc.sync.dma_start(
                        out=scales[(b, g, br)],
                        in_=zd_d[b, g, br].rearrange("c h w -> c (h w)"))
        # silu in channel-major layout: s = (z + bias) * sigmoid(z + bias)
        with tc.tile_pool(name="sg", bufs=2) as sgp:
            for b in range(BPC):
                for g in range(2):
                    for br in range(3):
                        zt = scales[(b, g, br)]
                        sg = sgp.tile([128, PX], BF16, tag="sg")
                        nc.scalar.activation(
                            sg, zt, AF.Sigmoid,
                            bias=dwb[:, g, br:br + 1], scale=1.0)
                        nc.vector.scalar_tensor_tensor(
                            zt, zt, dwb[:, g, br:br + 1], sg, OP.add, OP.mult)

        # persistent mid-pipeline tensors (opened after dw pools closed)
        sum3_p = ctx.enter_context(tc.tile_pool(name="sum3", bufs=4))
        sum3 = {}

        # ---- phase 3: sum/max over branches+channels, spatial attention ----
        with tc.tile_pool(name="p3", bufs=2) as p3, \
             tc.tile_pool(name="sp", bufs=6) as sp:
            for b in range(BPC):
                pads = {}
                for g in range(2):
                    s0, s1, s2 = (scales[(b, g, i)] for i in range(3))
                    t01 = p3.tile([128, PX], BF16, tag="tmp")
                    nc.vector.tensor_tensor(t01, s0, s1, OP.add)
                    s3 = sum3_p.tile([128, PX], BF16, tag="sum3")
                    nc.vector.tensor_tensor(s3, t01, s2, OP.add)
                    sum3[(b, g)] = s3
                    m01 = p3.tile([128, PX], BF16, tag="tmp")
                    nc.vector.tensor_tensor(m01, s0, s1, OP.max)
                    mx = p3.tile([128, PX], BF16, tag="mx")
                    nc.vector.tensor_tensor(mx, m01, s2, OP.max)
                    # reduce across the 128 partitions (channels)
                    ar = p3.tile([128, PX], BF16, tag="ar")
                    nc.gpsimd.partition_all_reduce(ar, s3, 128, RED.add)
                    mr = p3.tile([128, PX], BF16, tag="ar")
                    nc.gpsimd.partition_all_reduce(mr, mx, 128, RED.max)
                    pa = sp.tile([64, 64], BF16, tag="pads")
                    nc.sync.dma_start(out=pa, in_=ar[0:1, :])
                    pm = sp.tile([64, 64], BF16, tag="pads")
                    nc.sync.dma_start(out=pm, in_=mr[0:1, :])
                    pads[(g, "a")], pads[(g, "m")] = pa, pm
                # combine groups into [64, 64] maps (base partition 0)
                cmba = sp.tile([64, 64], F32, tag="cmba")
                cmbm = sp.tile([64, 64], F32, tag="cmbm")
                nc.vector.tensor_tensor(cmba, pads[(0, "a")], pads[(1, "a")], OP.add)
                nc.vector.tensor_tensor(cmbm, pads[(0, "m")], pads[(1, "m")], OP.max)
                # engines cannot address partition offsets != 0 mod 32, so
                # build h-shifted copies via DMA: sh[d][h, 1+w] = map[h+d, w]
                sh = {}
                for ci, cmb in ((0, cmba), (1, cmbm)):
                    for d in (-1, 0, 1):
                        t = sp.tile([64, 66], F32, tag="shift")
                        nc.vector.memset(t, 0.0)
                        n = 64 - abs(d)
                        nc.sync.dma_start(
                            out=t[max(0, -d):max(0, -d) + n, 1:65],
                            in_=cmb[max(0, d):max(0, d) + n, :])
                        sh[(ci, d)] = t
                # 3x3 conv over (avg, max) with folded taps
                acc2 = sp.tile([64, 64], F32, tag="acc2")
                first = True
                for ci, taps in ((0, tap_avg), (1, tap_max)):
                    for dh in range(3):
                        for dw in range(3):
                            src = sh[(ci, dh - 1)][:, dw:dw + 64]
                            wv = float(taps[dh, dw])
                            if first:
                                nc.vector.tensor_scalar_mul(acc2, src, wv)
                                first = False
                            else:
                                nc.vector.scalar_tensor_tensor(
                                    acc2, src, wv, acc2, OP.mult, OP.add)
                tu = sp.tile([64, 64], BF16, tag="tu")
                bu = sp.tile([64, 1], F32, tag="bu")
                nc.vector.memset(bu, bias_u)
                nc.scalar.activation(tu, acc2, AF.Sigmoid, bias=bu, scale=1.0)
                nc.sync.dma_start(out=st_d[b, :], in_=tu)

        # ---- phase 4: 1x1 conv (768->256) -> SiLU -> tanh-half ----
        th_p = ctx.enter_context(tc.tile_pool(name="th", bufs=4))
        th = {}
        with tc.tile_pool(name="ps", bufs=4, space="PSUM") as psp, \
             tc.tile_pool(name="p4", bufs=4) as p4:
            for b in range(BPC):
                for m in range(2):
                    tht = th_p.tile([128, PX], BF16, tag="th")
                    th[(b, m)] = tht
                for pxc in range(PX // PXC):
                    c0 = pxc * PXC
                    for m in range(2):
                        ps = psp.tile([128, PXC], F32, tag="ps")
                        for kc in range(6):
                            br, g = kc // 2, kc % 2
                            nc.tensor.matmul(
                                ps, aw[:, kc, m, :],
                                scales[(b, g, br)][:, c0:c0 + PXC],
                                start=(kc == 0), stop=(kc == 5))
                        # t = silu(ps + ba); attn = sigmoid(t)
                        sg1 = p4.tile([128, PXC], BF16, tag="sg1")
                        nc.scalar.activation(
                            sg1, ps, AF.Sigmoid, bias=ba[:, m:m + 1], scale=1.0)
                        tt = p4.tile([128, PXC], BF16, tag="tt")
                        nc.vector.scalar_tensor_tensor(
                            tt, ps, ba[:, m:m + 1], sg1, OP.add, OP.mult)
                        nc.scalar.activation(
                            th[(b, m)][:, c0:c0 + PXC], tt, AF.Sigmoid, scale=1.0)

        # ---- phase 6: out = 0.5*((th+1)*sum3 + (tu+1)*x) ----
        with tc.tile_pool(name="p6", bufs=4) as p6:
            for b in range(BPC):
                for g in range(2):
                    for pxc in range(PX // PXC):
                        c0 = pxc * PXC
                        r0 = c0 // 64
                        xa = p6.tile([128, 8, 64], F32, tag="xa")
                        nc.sync.dma_start(
                            out=xa,
                            in_=x_d[b, g * 128:(g + 1) * 128, r0:r0 + 8, :])
                        u1c = p6.tile([128, PXC], BF16, tag="u1c")
                        nc.sync.dma_start(
                            out=u1c,
                            in_=st_d[b:b + 1, c0:c0 + PXC].to_broadcast((128, PXC)))
                        q = p6.tile([128, PXC], F32, tag="q")
                        nc.vector.tensor_tensor(
                            q, u1c, xa.rearrange("p a b -> p (a b)"), OP.mult)
                        pp = p6.tile([128, PXC], F32, tag="pp")
                        nc.vector.tensor_tensor(
                            pp, th[(b, g)][:, c0:c0 + PXC],
                            sum3[(b, g)][:, c0:c0 + PXC], OP.mult)
                        ot = p6.tile([128, 8, 64], F32, tag="ot")
                        nc.vector.tensor_tensor(
                            ot.rearrange("p a b -> p (a b)"), pp, q, OP.add)
                        nc.sync.dma_start(
                            out=out_d[b, g * 128:(g + 1) * 128, r0:r0 + 8, :],
                            in_=ot)
    nc.compile()
    return nc


def _prep(inputs):
    """Host-side folding of BN/bias into conv weights; builds per-core input maps."""
    f = lambda a: np.asarray(a, dtype=np.float32)
    x = f(inputs["x"])
    dw_w = [f(w) for w in inputs["dw_w"]]
    dw_b, bn_g, bn_b = f(inputs["dw_b"]), f(inputs["bn_g"]), f(inputs["bn_b"])
    bn_m, bn_v = f(inputs["bn_m"]), f(inputs["bn_v"])
    aw, ab = f(inputs["aw"]), f(inputs["ab"])
    a_g, a_b = f(inputs["a_g"]), f(inputs["a_b"])
    a_m, a_v = f(inputs["a_m"]), f(inputs["a_v"])
    sq_w, sq_b = f(inputs["sq_w"]), f(inputs["sq_b"])

    s = bn_g / np.sqrt(bn_v + EPS)                  # (3, C)
    biasdw = (dw_b - bn_m) * s + bn_b               # (3, C)
    dwb = np.zeros((128, 2, 3), np.float32)
    for br in range(3):
        for g in range(2):
            dwb[:, g, br] = biasdw[br, g * 128:(g + 1) * 128]
    # banded Toeplitz stationaries [pair, t, c2, h_in, h_out]:
    # S[h_in, h_out] = w'[dh = h_in - h_out + p, delta] for each tap column delta
    sta = np.zeros((128, 15, 2, 64, 64), np.float32)
    t0 = 0
    for br, k in enumerate(KS):
        p = k // 2
        wf = dw_w[br][:, :, 0, :] * s[br][None, None, :]   # (k, k, C)
        for dlt in range(k):
            blk = sta[:, t0 + dlt]                           # (pair, c2, h_in, h_out)
            for dh in range(k):
                d = dh - p
                js = np.arange(max(0, -d), 64 - max(0, d))   # h_out
                wv = wf[dh, dlt, :].reshape(128, 2)          # (pair, c2)
                blk[:, :, js + d, js] = wv[:, :, None]
        t0 += k

    sa = a_g / np.sqrt(a_v + EPS)                   # (256,)
    awf = aw[0, 0] * sa[None, :]                    # (768, 256)
    awt = np.zeros((128, 6, 2, 128), np.float32)
    for kc in range(6):
        for m in range(2):
            awt[:, kc, m, :] = awf[kc * 128:(kc + 1) * 128, m * 128:(m + 1) * 128]
    ba2 = ((ab - a_m) * sa + a_b).reshape(2, 128).T.copy()   # (128, 2)

    tap_avg = sq_w[:, :, 0, 0] / 768.0
    tap_max = sq_w[:, :, 1, 0]
    bias_u = float(sq_b[0])

    import ml_dtypes
    awt_bf = awt.astype(ml_dtypes.bfloat16)
    sta_bf = sta.astype(ml_dtypes.bfloat16)
    in_maps = []
    for core in range(N_CORES):
        in_maps.append({
            "x": np.ascontiguousarray(x[core * BPC:(core + 1) * BPC]),
            "sta": sta_bf, "dwb": dwb, "aw": awt_bf, "ba": ba2,
        })
    return in_maps, (tap_avg, tap_max, bias_u)


def kernel(**inputs):
    in_maps, spatial = _prep(inputs)
    key = (spatial[0].tobytes(), spatial[1].tobytes(), spatial[2])
    if _CACHE.get("key") != key:
        _CACHE["nc"] = build(*spatial)
        _CACHE["key"] = key
    res = run_bass_kernel_spmd(_CACHE["nc"], in_maps, core_ids=list(range(N_CORES)))
    out = np.concatenate([r["out"] for r in res.results], axis=0)
    return np.ascontiguousarray(out, dtype=np.float32)


# revision 36
# speedup vs baseline: 3.2599x; 1.5085x over previous
"""CrossScaleConvolutionalAttentionDenoising Trainium2 Bass kernel.

Data-parallel over batch: B=16 images, 8 NeuronCores, 2 images per core.
All params replicated; BN folded into conv weights/biases on the host.
Layout: channels on partitions, pixels (h*64+w) on the free dim.
  - 3 depthwise convs (k=3,5,7) + bias -> SiLU          (branch scales)
  - 1x1 conv 768->256 (PE matmul) + bias -> SiLU -> "sigmoid" (channel attn)
  - channel mean/max of fused -> 3x3 conv (2->1) -> "sigmoid"  (spatial attn)
  - out = attn_w * (s0+s1+s2) + attn2 * x
sigmoid(z) is computed as 0.5*(tanh(z/2)+1) so the whole kernel uses only the
silu_and_others ACT table set (silu + tanh) -- no table reloads.
"""

from contextlib import ExitStack

import numpy as np

import concourse.bacc as bacc
import concourse.bass as bass
import concourse.bass_isa as bass_isa
import concourse.tile as tile
from concourse import mybir
from concourse.bass_utils import run_bass_kernel_spmd

N_CORES = 8
B, C, H, W = 16, 256, 64, 64
BPC = B // N_CORES  # images per core
KS = (3, 5, 7)
EPS = 1e-5
PX = H * W  # 4096
PXC = 512   # pixel chunk
F32 = mybir.dt.float32
BF16 = mybir.dt.bfloat16
AF = mybir.ActivationFunctionType
OP = mybir.AluOpType
RED = bass_isa.ReduceOp

_CACHE = {}


def _taps(k):
    p = k // 2
    for dh in range(k):
        for dw in range(k):
            yield 3 - p + dh, 3 - p + dw  # slice offsets into the 70x70 pad


def build(tap_avg, tap_max, bias_u):
    nc = bacc.Bacc("TRN2", target_bir_lowering=False, debug=False)
    x_d = nc.dram_tensor("x", [BPC, C, H, W], F32, kind="ExternalInput")
    # Toeplitz stationaries, laid out for one contiguous DMA per 4-pair chunk:
    # [q, c2, h_in, pl, t, h_out]
    sta_d = nc.dram_tensor("sta", [32, 2, 64, 4, 15, 64], BF16, kind="ExternalInput")
    dwb_d = nc.dram_tensor("dwb", [128, 2, 3], F32, kind="ExternalInput")
    aw_d = nc.dram_tensor("aw", [128, 6, 2, 128], BF16, kind="ExternalInput")
    ba_d = nc.dram_tensor("ba", [128, 2], F32, kind="ExternalInput")
    id_d = nc.dram_tensor("ident", [128, 128], BF16, kind="ExternalInput")
    out_d = nc.dram_tensor("out", [BPC, C, H, W], F32, kind="ExternalOutput")
    st_d = nc.dram_tensor("stage", [BPC, PX], BF16)   # spatial-attn sigmoid map
    st2_d = nc.dram_tensor("stage2", [BPC, PX], BF16)  # channel-max map
    # dw reassembly scratch, one tensor per (g, br) for fine dep tracking:
    # layout [ch, h, b, w]
    zd_d = {(g, br): nc.dram_tensor(f"zs_{g}_{br}", [128, H, BPC, W], BF16)
            for g in range(2) for br in range(3)}

    with tile.TileContext(nc) as tc, ExitStack() as ctx:
        const = ctx.enter_context(tc.tile_pool(name="const", bufs=1))
        dwb = const.tile([128, 2, 3], F32)
        nc.sync.dma_start(out=dwb, in_=dwb_d[:, :, :])
        aw = const.tile([128, 6, 2, 128], BF16)
        nc.sync.dma_start(out=aw, in_=aw_d[:, :, :, :])
        ba = const.tile([128, 2], F32)
        nc.sync.dma_start(out=ba, in_=ba_d[:, :])
        ident = const.tile([128, 128], BF16)
        nc.sync.dma_start(out=ident, in_=id_d[:, :])
        ones = const.tile([128, 1], BF16)
        nc.vector.memset(ones, 1.0)

        scl_p = ctx.enter_context(tc.tile_pool(name="scl", bufs=12))
        scales = {}
        for b in range(BPC):
            for g in range(2):
                for br in range(3):
                    zt = scl_p.tile([128, PX], BF16, tag="scl")
                    scales[(b, g, br)] = zt

        # ---- phase 1+2: depthwise convs on the PE (banded Toeplitz over h) ----
        with tc.tile_pool(name="xb", bufs=1) as xbp, \
             tc.tile_pool(name="stg", bufs=3) as stgp, \
             tc.tile_pool(name="zst", bufs=6) as zstp, \
             tc.tile_pool(name="psd", bufs=2, space="PSUM") as psd:
            xb = xbp.tile([128, BPC, 128, 70], BF16)
            nc.vector.memset(xb, 0.0)
            for b in range(BPC):
                for ph in range(2):  # split to stay under DMA descriptor limit
                    nc.gpsimd.dma_start(  # gpsimd DMA casts f32 -> bf16
                        out=xb[:, b, ph * 64:(ph + 1) * 64, 3:67],
                        in_=x_d[b, ph * 128:(ph + 1) * 128].rearrange(
                            "(pr c2) h w -> (c2 h) pr w", c2=2))
            tslot = {}  # (br, dlt) -> (t index, w offset)
            t0 = 0
            for br, k in enumerate(KS):
                p = k // 2
                for dlt in range(k):
                    tslot[(br, dlt)] = (t0 + dlt, 3 - p + dlt)
                t0 += k
            for q in range(32):  # 4 pairs per chunk
                stg = stgp.tile([128, 4, 15, 64], BF16, tag="stg")
                nc.gpsimd.dma_start(
                    out=stg.rearrange("p a b c -> p (a b c)"),
                    in_=sta_d[q].rearrange("c2 hi pl t ho -> (c2 hi) (pl t ho)"))
                pss = []
                for br in range(3):
                    pst = psd.tile([128, 512], F32, tag=f"ps{br}")
                    pss.append(pst)
                for pl in range(4):
                    pair = 4 * q + pl
                    for br, k in enumerate(KS):
                        for c2 in range(2):
                            lo = c2 * 64
                            for dlt in range(k):
                                t, off = tslot[(br, dlt)]
                                nc.tensor.matmul(
                                    pss[br][lo:lo + 64, pl * 128:(pl + 1) * 128],
                                    stg[lo:lo + 64, pl, t, :],
                                    xb[lo:lo + 64, :, pair, off:off + 64],
                                    start=(dlt == 0), stop=(dlt == k - 1),
                                    tile_position=(lo, lo))
                # PSUM -> bf16 staging -> DRAM in channel-major order
                g, pb = q // 16, (q % 16) * 4
                for br in range(3):
                    zs = zstp.tile([128, 4, BPC, 64], BF16, tag="zst")
                    if br % 2 == 0:
                        nc.vector.tensor_copy(
                            zs.rearrange("p a b c -> p (a b c)"), pss[br])
                    else:
                        nc.scalar.copy(
                            zs.rearrange("p a b c -> p (a b c)"), pss[br])
                    dst = zd_d[(g, br)][2 * pb:2 * pb + 8].rearrange(
                        "(pl c2) h b w -> (c2 h) pl (b w)", c2=2)
                    eng = nc.sync if br % 2 == 0 else nc.scalar
                    eng.dma_start(
                        out=dst, in_=zs.rearrange("p pl b w -> p pl (b w)"))
        # gather z tiles (channel-major) + silu: s = (z+bias)*sigmoid(z+bias)
        with tc.tile_pool(name="sg", bufs=3) as sgp:
            for g in range(2):
                for br in range(3):
                    for b in range(BPC):
                        eng = nc.sync if (br + b) % 2 == 0 else nc.scalar
                        eng.dma_start(
                            out=scales[(b, g, br)].rearrange(
                                "c (h w) -> c h w", h=64),
                            in_=zd_d[(g, br)][:, :, b, :])
            for b in range(BPC):
                for g in range(2):
                    for br in range(3):
                        zt = scales[(b, g, br)]
                        sg = sgp.tile([128, PX], BF16, tag="sg")
                        nc.scalar.activation(
                            sg, zt, AF.Sigmoid,
                            bias=dwb[:, g, br:br + 1], scale=1.0)
                        nc.vector.scalar_tensor_tensor(
                            zt, zt, dwb[:, g, br:br + 1], sg, OP.add, OP.mult)

        # ---- phase 3: branch sum/max, channel stats (PE-based), spatial attn ----
        sum3_p = ctx.enter_context(tc.tile_pool(name="sum3", bufs=4))
        sum3 = {}
        with tc.tile_pool(name="p3", bufs=2) as p3, \
             tc.tile_pool(name="mps", bufs=4, space="PSUM") as mps, \
             tc.tile_pool(name="avp", bufs=2) as avp, \
             tc.tile_pool(name="sp", bufs=6) as sp:
            for b in range(BPC):
                mxs = {}
                for g in range(2):
                    s0, s1, s2 = (scales[(b, g, i)] for i in range(3))
                    t01 = p3.tile([128, PX], BF16, tag="tmp")
                    nc.vector.tensor_tensor(t01, s0, s1, OP.add)
                    s3 = sum3_p.tile([128, PX], BF16, tag="sum3")
                    nc.vector.tensor_tensor(s3, t01, s2, OP.add)
                    sum3[(b, g)] = s3
                    m01 = p3.tile([128, PX], BF16, tag="tmp")
                    nc.vector.tensor_tensor(m01, s0, s1, OP.max)
                    mx = p3.tile([128, PX], BF16, tag="mx")
                    nc.vector.tensor_tensor(mx, m01, s2, OP.max)
                    mxs[g] = mx
                # channel sum via ones-matmul (accumulate g0+g1), px chunks of 512
                avrow = avp.tile([1, PX], BF16, tag="avrow")
                for ch in range(8):
                    c0 = ch * 512
                    av = mps.tile([1, 512], F32, tag="av")
                    nc.tensor.matmul(av, ones, sum3[(b, 0)][:, c0:c0 + 512],
                                     start=True, stop=False)
                    nc.tensor.matmul(av, ones, sum3[(b, 1)][:, c0:c0 + 512],
                                     start=False, stop=True)
                    nc.scalar.copy(avrow[:, c0:c0 + 512], av)
                # channel max via PE transpose + free-dim reduce, px chunks of 128
                maxc = sp.tile([128, 32], BF16, tag="maxc")
                for ch in range(32):
                    c0 = ch * 128
                    mt = mps.tile([128, 256], BF16, tag="mt")
                    nc.tensor.transpose(
                        mt[:, 0:128], mxs[0][:, c0:c0 + 128], ident)
                    nc.tensor.transpose(
                        mt[:, 128:256], mxs[1][:, c0:c0 + 128], ident)
                    nc.vector.tensor_reduce(
                        maxc[:, ch:ch + 1], mt, mybir.AxisListType.X, OP.max)
                # maxc[p, ch] holds px = ch*128 + p -> write px-major to DRAM
                mslice = st2_d[b]
                nc.sync.dma_start(
                    out=bass.AP(tensor=mslice.tensor, offset=mslice.offset,
                                ap=[[1, 128], [128, 32]]),
                    in_=maxc)
                cmbm = sp.tile([64, 64], BF16, tag="cmbm")
                nc.scalar.dma_start(
                    out=cmbm, in_=st2_d[b].rearrange("(h w) -> h w", h=64))
                cmba = sp.tile([64, 64], BF16, tag="cmba")
                nc.sync.dma_start(out=cmba, in_=avrow[0:1, :])
                # h-shifted copies via DMA: sh[d][h, 1+w] = map[h+d, w]
                sh = {}
                for ci, cmb in ((0, cmba), (1, cmbm)):
                    for d in (-1, 0, 1):
                        t = sp.tile([64, 66], BF16, tag="shift")
                        nc.vector.memset(t, 0.0)
                        n = 64 - abs(d)
                        eng = nc.sync if ci == 0 else nc.scalar
                        eng.dma_start(
                            out=t[max(0, -d):max(0, -d) + n, 1:65],
                            in_=cmb[max(0, d):max(0, d) + n, :])
                        sh[(ci, d)] = t
                # 3x3 conv over (avg, max) with folded taps
                acc2 = sp.tile([64, 64], F32, tag="acc2")
                first = True
                for ci, taps in ((0, tap_avg), (1, tap_max)):
                    for dh in range(3):
                        for dw in range(3):
                            src = sh[(ci, dh - 1)][:, dw:dw + 64]
                            wv = float(taps[dh, dw])
                            if first:
                                nc.vector.tensor_scalar_mul(acc2, src, wv)
                                first = False
                            else:
                                nc.vector.scalar_tensor_tensor(
                                    acc2, src, wv, acc2, OP.mult, OP.add)
                tu = sp.tile([64, 64], BF16, tag="tu")
                bu = sp.tile([64, 1], F32, tag="bu")
                nc.vector.memset(bu, bias_u)
                nc.scalar.activation(tu, acc2, AF.Sigmoid, bias=bu, scale=1.0)
                nc.sync.dma_start(out=st_d[b, :], in_=tu)

        # ---- phase 4+6 merged per image: 1x1 conv attn, then final combine ----
        th_p = ctx.enter_context(tc.tile_pool(name="th", bufs=4))
        with tc.tile_pool(name="ps", bufs=4, space="PSUM") as psp, \
             tc.tile_pool(name="p4", bufs=4) as p4, \
             tc.tile_pool(name="p6", bufs=2) as p6:
            for b in range(BPC):
                th = {}
                for m in range(2):
                    tht = th_p.tile([128, PX], BF16, tag="th")
                    th[m] = tht
                for pxc in range(PX // PXC):
                    c0 = pxc * PXC
                    for m in range(2):
                        ps = psp.tile([128, PXC], F32, tag="ps")
                        for kc in range(6):
                            br, g = kc // 2, kc % 2
                            nc.tensor.matmul(
                                ps, aw[:, kc, m, :],
                                scales[(b, g, br)][:, c0:c0 + PXC],
                                start=(kc == 0), stop=(kc == 5))
                        # t = silu(ps + ba); attn = sigmoid(t)
                        sg1 = p4.tile([128, PXC], BF16, tag="sg1")
                        nc.scalar.activation(
                            sg1, ps, AF.Sigmoid, bias=ba[:, m:m + 1], scale=1.0)
                        tt = p4.tile([128, PXC], BF16, tag="tt")
                        nc.vector.scalar_tensor_tensor(
                            tt, ps, ba[:, m:m + 1], sg1, OP.add, OP.mult)
                        nc.scalar.activation(
                            th[m][:, c0:c0 + PXC], tt, AF.Sigmoid, scale=1.0)
                for g in range(2):
                    for pxc in range(PX // 1024):
                        c0 = pxc * 1024
                        r0 = c0 // 64
                        xa = p6.tile([128, 16, 64], F32, tag="xa")
                        nc.sync.dma_start(
                            out=xa,
                            in_=x_d[b, g * 128:(g + 1) * 128, r0:r0 + 16, :])
                        u1c = p6.tile([128, 1024], BF16, tag="u1c")
                        nc.scalar.dma_start(
                            out=u1c,
                            in_=st_d[b:b + 1, c0:c0 + 1024].to_broadcast((128, 1024)))
                        q6 = p6.tile([128, 1024], F32, tag="q6")
                        nc.vector.tensor_tensor(
                            q6, u1c, xa.rearrange("p a b -> p (a b)"), OP.mult)
                        pp = p6.tile([128, 1024], BF16, tag="pp")
                        nc.vector.tensor_tensor(
                            pp, th[g][:, c0:c0 + 1024],
                            sum3[(b, g)][:, c0:c0 + 1024], OP.mult)
                        ot = p6.tile([128, 16, 64], F32, tag="ot")
                        nc.vector.tensor_tensor(
                            ot.rearrange("p a b -> p (a b)"), pp, q6, OP.add)
                        nc.sync.dma_start(
                            out=out_d[b, g * 128:(g + 1) * 128, r0:r0 + 16, :],
                            in_=ot)
    nc.compile()
    return nc


def _prep(inputs):
    """Host-side folding of BN/bias into conv weights; builds per-core input maps."""
    f = lambda a: np.asarray(a, dtype=np.float32)
    x = f(inputs["x"])
    dw_w = [f(w) for w in inputs["dw_w"]]
    dw_b, bn_g, bn_b = f(inputs["dw_b"]), f(inputs["bn_g"]), f(inputs["bn_b"])
    bn_m, bn_v = f(inputs["bn_m"]), f(inputs["bn_v"])
    aw, ab = f(inputs["aw"]), f(inputs["ab"])
    a_g, a_b = f(inputs["a_g"]), f(inputs["a_b"])
    a_m, a_v = f(inputs["a_m"]), f(inputs["a_v"])
    sq_w, sq_b = f(inputs["sq_w"]), f(inputs["sq_b"])

    s = bn_g / np.sqrt(bn_v + EPS)                  # (3, C)
    biasdw = (dw_b - bn_m) * s + bn_b               # (3, C)
    dwb = np.zeros((128, 2, 3), np.float32)
    for br in range(3):
        for g in range(2):
            dwb[:, g, br] = biasdw[br, g * 128:(g + 1) * 128]
    # banded Toeplitz stationaries [pair, t, c2, h_in, h_out]:
    # S[h_in, h_out] = w'[dh = h_in - h_out + p, delta] for each tap column delta
    sta = np.zeros((128, 15, 2, 64, 64), np.float32)
    t0 = 0
    for br, k in enumerate(KS):
        p = k // 2
        wf = dw_w[br][:, :, 0, :] * s[br][None, None, :]   # (k, k, C)
        for dlt in range(k):
            blk = sta[:, t0 + dlt]                           # (pair, c2, h_in, h_out)
            for dh in range(k):
                d = dh - p
                js = np.arange(max(0, -d), 64 - max(0, d))   # h_out
                wv = wf[dh, dlt, :].reshape(128, 2)          # (pair, c2)
                blk[:, :, js + d, js] = wv[:, :, None]
        t0 += k

    sa = a_g / np.sqrt(a_v + EPS)                   # (256,)
    awf = aw[0, 0] * sa[None, :]                    # (768, 256)
    awt = np.zeros((128, 6, 2, 128), np.float32)
    for kc in range(6):
        for m in range(2):
            awt[:, kc, m, :] = awf[kc * 128:(kc + 1) * 128, m * 128:(m + 1) * 128]
    ba2 = ((ab - a_m) * sa + a_b).reshape(2, 128).T.copy()   # (128, 2)

    tap_avg = sq_w[:, :, 0, 0] / 768.0
    tap_max = sq_w[:, :, 1, 0]
    bias_u = float(sq_b[0])

    import ml_dtypes
    awt_bf = awt.astype(ml_dtypes.bfloat16)
    # reorder for one contiguous DMA per 4-pair chunk: [q, c2, h_in, pl, t, h_out]
    sta2 = np.ascontiguousarray(
        sta.reshape(32, 4, 15, 2, 64, 64).transpose(0, 3, 4, 1, 2, 5))
    sta_bf = sta2.astype(ml_dtypes.bfloat16)
    ident = np.eye(128, dtype=np.float32).astype(ml_dtypes.bfloat16)
    in_maps = []
    for core in range(N_CORES):
        in_maps.append({
            "x": np.ascontiguousarray(x[core * BPC:(core + 1) * BPC]),
            "sta": sta_bf, "dwb": dwb, "aw": awt_bf, "ba": ba2,
            "ident": ident,
        })
    return in_maps, (tap_avg, tap_max, bias_u)


def kernel(**inputs):
    in_maps, spatial = _prep(inputs)
    key = (spatial[0].tobytes(), spatial[1].tobytes(), spatial[2])
    if _CACHE.get("key") != key:
        _CACHE["nc"] = build(*spatial)
        _CACHE["key"] = key
    res = run_bass_kernel_spmd(_CACHE["nc"], in_maps, core_ids=list(range(N_CORES)))
    out = np.concatenate([r["out"] for r in res.results], axis=0)
    return np.ascontiguousarray(out, dtype=np.float32)


# revision 37
# speedup vs baseline: 3.3590x; 1.0304x over previous
"""CrossScaleConvolutionalAttentionDenoising Trainium2 Bass kernel.

Data-parallel over batch: B=16 images, 8 NeuronCores, 2 images per core.
All params replicated; BN folded into conv weights/biases on the host.
Layout: channels on partitions, pixels (h*64+w) on the free dim.
  - 3 depthwise convs (k=3,5,7) + bias -> SiLU          (branch scales)
  - 1x1 conv 768->256 (PE matmul) + bias -> SiLU -> "sigmoid" (channel attn)
  - channel mean/max of fused -> 3x3 conv (2->1) -> "sigmoid"  (spatial attn)
  - out = attn_w * (s0+s1+s2) + attn2 * x
sigmoid(z) is computed as 0.5*(tanh(z/2)+1) so the whole kernel uses only the
silu_and_others ACT table set (silu + tanh) -- no table reloads.
"""

from contextlib import ExitStack

import numpy as np

import concourse.bacc as bacc
import concourse.bass as bass
import concourse.bass_isa as bass_isa
import concourse.tile as tile
from concourse import mybir
from concourse.bass_utils import run_bass_kernel_spmd

N_CORES = 8
B, C, H, W = 16, 256, 64, 64
BPC = B // N_CORES  # images per core
KS = (3, 5, 7)
EPS = 1e-5
PX = H * W  # 4096
PXC = 512   # pixel chunk
F32 = mybir.dt.float32
BF16 = mybir.dt.bfloat16
AF = mybir.ActivationFunctionType
OP = mybir.AluOpType
RED = bass_isa.ReduceOp

_CACHE = {}


def _taps(k):
    p = k // 2
    for dh in range(k):
        for dw in range(k):
            yield 3 - p + dh, 3 - p + dw  # slice offsets into the 70x70 pad


def build(tap_avg, tap_max, bias_u):
    nc = bacc.Bacc("TRN2", target_bir_lowering=False, debug=False)
    x_d = nc.dram_tensor("x", [BPC, C, H, W], F32, kind="ExternalInput")
    # Toeplitz stationaries, laid out for one contiguous DMA per 4-pair chunk:
    # [q, c2, h_in, pl, t, h_out]
    sta_d = nc.dram_tensor("sta", [32, 2, 64, 4, 15, 64], BF16, kind="ExternalInput")
    dwb_d = nc.dram_tensor("dwb", [128, 2, 3], F32, kind="ExternalInput")
    aw_d = nc.dram_tensor("aw", [128, 6, 2, 128], BF16, kind="ExternalInput")
    ba_d = nc.dram_tensor("ba", [128, 2], F32, kind="ExternalInput")
    id_d = nc.dram_tensor("ident", [128, 128], BF16, kind="ExternalInput")
    out_d = nc.dram_tensor("out", [BPC, C, H, W], F32, kind="ExternalOutput")
    st_d = nc.dram_tensor("stage", [BPC, PX], BF16)   # spatial-attn sigmoid map
    st2_d = nc.dram_tensor("stage2", [BPC, PX], BF16)  # channel-max map
    # dw reassembly scratch, one tensor per (g, br) for fine dep tracking:
    # layout [ch, h, b, w]
    zd_d = {(g, br): nc.dram_tensor(f"zs_{g}_{br}", [128, H, BPC, W], BF16)
            for g in range(2) for br in range(3)}

    with tile.TileContext(nc) as tc, ExitStack() as ctx:
        const = ctx.enter_context(tc.tile_pool(name="const", bufs=1))
        dwb = const.tile([128, 2, 3], F32)
        nc.sync.dma_start(out=dwb, in_=dwb_d[:, :, :])
        aw = const.tile([128, 6, 2, 128], BF16)
        nc.sync.dma_start(out=aw, in_=aw_d[:, :, :, :])
        ba = const.tile([128, 2], F32)
        nc.sync.dma_start(out=ba, in_=ba_d[:, :])
        ident = const.tile([128, 128], BF16)
        nc.sync.dma_start(out=ident, in_=id_d[:, :])
        ones = const.tile([128, 1], BF16)
        nc.vector.memset(ones, 1.0)

        scl_p = ctx.enter_context(tc.tile_pool(name="scl", bufs=12))
        scales = {}
        for b in range(BPC):
            for g in range(2):
                for br in range(3):
                    zt = scl_p.tile([128, PX], BF16, tag="scl")
                    scales[(b, g, br)] = zt

        # ---- phase 1+2: depthwise convs on the PE (banded Toeplitz over h) ----
        with tc.tile_pool(name="xb", bufs=1) as xbp, \
             tc.tile_pool(name="stg", bufs=3) as stgp, \
             tc.tile_pool(name="zst", bufs=6) as zstp, \
             tc.tile_pool(name="psd", bufs=2, space="PSUM") as psd:
            xb = xbp.tile([128, BPC, 128, 70], BF16)
            nc.vector.memset(xb, 0.0)
            for b in range(BPC):
                for ph in range(2):  # split to stay under DMA descriptor limit
                    nc.gpsimd.dma_start(  # gpsimd DMA casts f32 -> bf16
                        out=xb[:, b, ph * 64:(ph + 1) * 64, 3:67],
                        in_=x_d[b, ph * 128:(ph + 1) * 128].rearrange(
                            "(pr c2) h w -> (c2 h) pr w", c2=2))
            tslot = {}  # (br, dlt) -> (t index, w offset)
            t0 = 0
            for br, k in enumerate(KS):
                p = k // 2
                for dlt in range(k):
                    tslot[(br, dlt)] = (t0 + dlt, 3 - p + dlt)
                t0 += k
            for q in range(32):  # 4 pairs per chunk
                stg = stgp.tile([128, 4, 15, 64], BF16, tag="stg")
                nc.gpsimd.dma_start(
                    out=stg.rearrange("p a b c -> p (a b c)"),
                    in_=sta_d[q].rearrange("c2 hi pl t ho -> (c2 hi) (pl t ho)"))
                pss = []
                for br in range(3):
                    pst = psd.tile([128, 512], F32, tag=f"ps{br}")
                    pss.append(pst)
                for pl in range(4):
                    pair = 4 * q + pl
                    for br, k in enumerate(KS):
                        for c2 in range(2):
                            lo = c2 * 64
                            for dlt in range(k):
                                t, off = tslot[(br, dlt)]
                                nc.tensor.matmul(
                                    pss[br][lo:lo + 64, pl * 128:(pl + 1) * 128],
                                    stg[lo:lo + 64, pl, t, :],
                                    xb[lo:lo + 64, :, pair, off:off + 64],
                                    start=(dlt == 0), stop=(dlt == k - 1),
                                    tile_position=(lo, lo))
                # PSUM -> bf16 staging -> DRAM in channel-major order
                g, pb = q // 16, (q % 16) * 4
                for br in range(3):
                    zs = zstp.tile([128, 4, BPC, 64], BF16, tag="zst")
                    nc.vector.tensor_copy(
                        zs.rearrange("p a b c -> p (a b c)"), pss[br])
                    dst = zd_d[(g, br)][2 * pb:2 * pb + 8].rearrange(
                        "(pl c2) h b w -> (c2 h) pl (b w)", c2=2)
                    nc.sync.dma_start(
                        out=dst, in_=zs.rearrange("p pl b w -> p pl (b w)"))
        # gather z tiles (channel-major) + silu: s = (z+bias)*sigmoid(z+bias)
        with tc.tile_pool(name="sg", bufs=3) as sgp:
            for g in range(2):
                for br in range(3):
                    for b in range(BPC):
                        nc.sync.dma_start(
                            out=scales[(b, g, br)].rearrange(
                                "c (h w) -> c h w", h=64),
                            in_=zd_d[(g, br)][:, :, b, :])
            for b in range(BPC):
                for g in range(2):
                    for br in range(3):
                        zt = scales[(b, g, br)]
                        sg = sgp.tile([128, PX], BF16, tag="sg")
                        nc.scalar.activation(
                            sg, zt, AF.Sigmoid,
                            bias=dwb[:, g, br:br + 1], scale=1.0)
                        nc.vector.scalar_tensor_tensor(
                            zt, zt, dwb[:, g, br:br + 1], sg, OP.add, OP.mult)

        # ---- phase 3: branch sum/max, channel stats (PE-based), spatial attn ----
        sum3_p = ctx.enter_context(tc.tile_pool(name="sum3", bufs=4))
        sum3 = {}
        with tc.tile_pool(name="p3", bufs=2) as p3, \
             tc.tile_pool(name="mps", bufs=4, space="PSUM") as mps, \
             tc.tile_pool(name="avp", bufs=2) as avp, \
             tc.tile_pool(name="sp", bufs=6) as sp:
            for b in range(BPC):
                mxs = {}
                for g in range(2):
                    s0, s1, s2 = (scales[(b, g, i)] for i in range(3))
                    t01 = p3.tile([128, PX], BF16, tag="tmp")
                    nc.vector.tensor_tensor(t01, s0, s1, OP.add)
                    s3 = sum3_p.tile([128, PX], BF16, tag="sum3")
                    nc.vector.tensor_tensor(s3, t01, s2, OP.add)
                    sum3[(b, g)] = s3
                    m01 = p3.tile([128, PX], BF16, tag="tmp")
                    nc.vector.tensor_tensor(m01, s0, s1, OP.max)
                    mx = p3.tile([128, PX], BF16, tag="mx")
                    nc.vector.tensor_tensor(mx, m01, s2, OP.max)
                    mxs[g] = mx
                # channel sum via ones-matmul (accumulate g0+g1), px chunks of 512
                avrow = avp.tile([1, PX], BF16, tag="avrow")
                for ch in range(8):
                    c0 = ch * 512
                    av = mps.tile([1, 512], F32, tag="av")
                    nc.tensor.matmul(av, ones, sum3[(b, 0)][:, c0:c0 + 512],
                                     start=True, stop=False)
                    nc.tensor.matmul(av, ones, sum3[(b, 1)][:, c0:c0 + 512],
                                     start=False, stop=True)
                    nc.scalar.copy(avrow[:, c0:c0 + 512], av)
                # channel max via PE transpose + free-dim reduce, px chunks of 128
                maxc = sp.tile([128, 32], BF16, tag="maxc")
                for ch in range(32):
                    c0 = ch * 128
                    mt = mps.tile([128, 256], BF16, tag="mt")
                    nc.tensor.transpose(
                        mt[:, 0:128], mxs[0][:, c0:c0 + 128], ident)
                    nc.tensor.transpose(
                        mt[:, 128:256], mxs[1][:, c0:c0 + 128], ident)
                    nc.vector.tensor_reduce(
                        maxc[:, ch:ch + 1], mt, mybir.AxisListType.X, OP.max)
                # maxc[p, ch] holds px = ch*128 + p -> write px-major to DRAM
                mslice = st2_d[b]
                nc.sync.dma_start(
                    out=bass.AP(tensor=mslice.tensor, offset=mslice.offset,
                                ap=[[1, 128], [128, 32]]),
                    in_=maxc)
                cmbm = sp.tile([64, 64], BF16, tag="cmbm")
                nc.scalar.dma_start(
                    out=cmbm, in_=st2_d[b].rearrange("(h w) -> h w", h=64))
                cmba = sp.tile([64, 64], BF16, tag="cmba")
                nc.sync.dma_start(out=cmba, in_=avrow[0:1, :])
                # h-shifted copies via DMA: sh[d][h, 1+w] = map[h+d, w]
                sh = {}
                for ci, cmb in ((0, cmba), (1, cmbm)):
                    for d in (-1, 0, 1):
                        t = sp.tile([64, 66], BF16, tag="shift")
                        nc.vector.memset(t, 0.0)
                        n = 64 - abs(d)
                        eng = nc.sync if ci == 0 else nc.scalar
                        eng.dma_start(
                            out=t[max(0, -d):max(0, -d) + n, 1:65],
                            in_=cmb[max(0, d):max(0, d) + n, :])
                        sh[(ci, d)] = t
                # 3x3 conv over (avg, max) with folded taps
                acc2 = sp.tile([64, 64], F32, tag="acc2")
                first = True
                for ci, taps in ((0, tap_avg), (1, tap_max)):
                    for dh in range(3):
                        for dw in range(3):
                            src = sh[(ci, dh - 1)][:, dw:dw + 64]
                            wv = float(taps[dh, dw])
                            if first:
                                nc.vector.tensor_scalar_mul(acc2, src, wv)
                                first = False
                            else:
                                nc.vector.scalar_tensor_tensor(
                                    acc2, src, wv, acc2, OP.mult, OP.add)
                tu = sp.tile([64, 64], BF16, tag="tu")
                bu = sp.tile([64, 1], F32, tag="bu")
                nc.vector.memset(bu, bias_u)
                nc.scalar.activation(tu, acc2, AF.Sigmoid, bias=bu, scale=1.0)
                nc.sync.dma_start(out=st_d[b, :], in_=tu)

        # ---- phase 4+6 merged per image: 1x1 conv attn, then final combine ----
        th_p = ctx.enter_context(tc.tile_pool(name="th", bufs=4))
        with tc.tile_pool(name="ps", bufs=4, space="PSUM") as psp, \
             tc.tile_pool(name="p4", bufs=4) as p4, \
             tc.tile_pool(name="p6", bufs=2) as p6:
            for b in range(BPC):
                th = {}
                for m in range(2):
                    tht = th_p.tile([128, PX], BF16, tag="th")
                    th[m] = tht
                for pxc in range(PX // PXC):
                    c0 = pxc * PXC
                    for m in range(2):
                        ps = psp.tile([128, PXC], F32, tag="ps")
                        for kc in range(6):
                            br, g = kc // 2, kc % 2
                            nc.tensor.matmul(
                                ps, aw[:, kc, m, :],
                                scales[(b, g, br)][:, c0:c0 + PXC],
                                start=(kc == 0), stop=(kc == 5))
                        # t = silu(ps + ba); attn = sigmoid(t)
                        sg1 = p4.tile([128, PXC], BF16, tag="sg1")
                        nc.scalar.activation(
                            sg1, ps, AF.Sigmoid, bias=ba[:, m:m + 1], scale=1.0)
                        tt = p4.tile([128, PXC], BF16, tag="tt")
                        nc.vector.scalar_tensor_tensor(
                            tt, ps, ba[:, m:m + 1], sg1, OP.add, OP.mult)
                        nc.scalar.activation(
                            th[m][:, c0:c0 + PXC], tt, AF.Sigmoid, scale=1.0)
                for g in range(2):
                    for pxc in range(PX // 1024):
                        c0 = pxc * 1024
                        r0 = c0 // 64
                        xa = p6.tile([128, 16, 64], BF16, tag="xa")
                        nc.gpsimd.dma_start(
                            out=xa,
                            in_=x_d[b, g * 128:(g + 1) * 128, r0:r0 + 16, :])
                        u1c = p6.tile([128, 1024], BF16, tag="u1c")
                        nc.scalar.dma_start(
                            out=u1c,
                            in_=st_d[b:b + 1, c0:c0 + 1024].to_broadcast((128, 1024)))
                        q6 = p6.tile([128, 1024], BF16, tag="q6")
                        nc.vector.tensor_tensor(
                            q6, u1c, xa.rearrange("p a b -> p (a b)"), OP.mult)
                        pp = p6.tile([128, 1024], BF16, tag="pp")
                        nc.vector.tensor_tensor(
                            pp, th[g][:, c0:c0 + 1024],
                            sum3[(b, g)][:, c0:c0 + 1024], OP.mult)
                        ot = p6.tile([128, 16, 64], F32, tag="ot")
                        nc.vector.tensor_tensor(
                            ot.rearrange("p a b -> p (a b)"), pp, q6, OP.add)
                        nc.sync.dma_start(
                            out=out_d[b, g * 128:(g + 1) * 128, r0:r0 + 16, :],
                            in_=ot)
    nc.compile()
    return nc


def _prep(inputs):
    """Host-side folding of BN/bias into conv weights; builds per-core input maps."""
    f = lambda a: np.asarray(a, dtype=np.float32)
    x = f(inputs["x"])
    dw_w = [f(w) for w in inputs["dw_w"]]
    dw_b, bn_g, bn_b = f(inputs["dw_b"]), f(inputs["bn_g"]), f(inputs["bn_b"])
    bn_m, bn_v = f(inputs["bn_m"]), f(inputs["bn_v"])
    aw, ab = f(inputs["aw"]), f(inputs["ab"])
    a_g, a_b = f(inputs["a_g"]), f(inputs["a_b"])
    a_m, a_v = f(inputs["a_m"]), f(inputs["a_v"])
    sq_w, sq_b = f(inputs["sq_w"]), f(inputs["sq_b"])

    s = bn_g / np.sqrt(bn_v + EPS)                  # (3, C)
    biasdw = (dw_b - bn_m) * s + bn_b               # (3, C)
    dwb = np.zeros((128, 2, 3), np.float32)
    for br in range(3):
        for g in range(2):
            dwb[:, g, br] = biasdw[br, g * 128:(g + 1) * 128]
    # banded Toeplitz stationaries [pair, t, c2, h_in, h_out]:
    # S[h_in, h_out] = w'[dh = h_in - h_out + p, delta] for each tap column delta
    sta = np.zeros((128, 15, 2, 64, 64), np.float32)
    t0 = 0
    for br, k in enumerate(KS):
        p = k // 2
        wf = dw_w[br][:, :, 0, :] * s[br][None, None, :]   # (k, k, C)
        for dlt in range(k):
            blk = sta[:, t0 + dlt]                           # (pair, c2, h_in, h_out)
            for dh in range(k):
                d = dh - p
                js = np.arange(max(0, -d), 64 - max(0, d))   # h_out
                wv = wf[dh, dlt, :].reshape(128, 2)          # (pair, c2)
                blk[:, :, js + d, js] = wv[:, :, None]
        t0 += k

    sa = a_g / np.sqrt(a_v + EPS)                   # (256,)
    awf = aw[0, 0] * sa[None, :]                    # (768, 256)
    awt = np.zeros((128, 6, 2, 128), np.float32)
    for kc in range(6):
        for m in range(2):
            awt[:, kc, m, :] = awf[kc * 128:(kc + 1) * 128, m * 128:(m + 1) * 128]
    ba2 = ((ab - a_m) * sa + a_b).reshape(2, 128).T.copy()   # (128, 2)

    tap_avg = sq_w[:, :, 0, 0] / 768.0
    tap_max = sq_w[:, :, 1, 0]
    bias_u = float(sq_b[0])

    import ml_dtypes
    awt_bf = awt.astype(ml_dtypes.bfloat16)
    # reorder for one contiguous DMA per 4-pair chunk: [q, c2, h_in, pl, t, h_out]
    sta2 = np.ascontiguousarray(
        sta.reshape(32, 4, 15, 2, 64, 64).transpose(0, 3, 4, 1, 2, 5))
    sta_bf = sta2.astype(ml_dtypes.bfloat16)
    ident = np.eye(128, dtype=np.float32).astype(ml_dtypes.bfloat16)
    in_maps = []
    for core in range(N_CORES):
        in_maps.append({
            "x": np.ascontiguousarray(x[core * BPC:(core + 1) * BPC]),
            "sta": sta_bf, "dwb": dwb, "aw": awt_bf, "ba": ba2,
            "ident": ident,
        })
    return in_maps, (tap_avg, tap_max, bias_u)


def kernel(**inputs):
    in_maps, spatial = _prep(inputs)
    key = (spatial[0].tobytes(), spatial[1].tobytes(), spatial[2])
    if _CACHE.get("key") != key:
        _CACHE["nc"] = build(*spatial)
        _CACHE["key"] = key
    res = run_bass_kernel_spmd(_CACHE["nc"], in_maps, core_ids=list(range(N_CORES)))
    out = np.concatenate([r["out"] for r in res.results], axis=0)
    return np.ascontiguousarray(out, dtype=np.float32)
